# revision 1
# baseline (speedup 1.0000x reference)
"""GwcVolumeCostProcessor Trainium2 kernel.

Builds the groupwise-correlation + concat cost volume:
  out[1, 64, 48, 128, 240] f32 from
  ref_gwc/tgt_gwc [1, 320, 128, 240] and ref_concat/tgt_concat [1, 12, 128, 240].

Sharding: H axis (128 = 8 cores x 16 rows). The disparity shift is along W
only, so each core needs just its own 16-row slice of every input.

Per-core pipeline (for each disparity d, descending):
  - DVE: prod[c,h,w] = ref_bf16[c,h,w+d] * tgt_bf16[c,h,w]   (bf16, 2x mode)
  - PE : block-diagonal [128,16] bf16 matmul reduces groups of 8 channels
         (mean) into PSUM
  - ACT: drains PSUM -> staging (fp32)
  - DMA: staging -> DRAM gwc channels; concat channels DMA'd straight from
         SBUF inputs with a zero-buffer for the w<d strips.
Staging slots are fully zeroed once; descending-d order keeps the w<d strip
zero without per-d memsets.
"""

import numpy as np
import ml_dtypes

C = 320          # gwc channels
G = 40           # groups
CPG = 8          # channels per group
D = 48           # disparity bins
H = 128          # full height
W = 240          # width
CC = 12          # concat channels per tensor
COUT = G + 2 * CC  # 64 output channels
NCORES = 8
HS = H // NCORES  # 16 rows per core

# channel tiles on partitions: (start, count, psum_base_partition, out_group_count)
# psum base partitions must be 32-aligned (PE col_grp constraint), so the
# three group blocks land sparsely at psum/staging partitions 0, 32, 64.
CH_TILES = [(0, 128, 0, 16), (128, 128, 32, 16), (256, 64, 64, 8)]
# (psum_row_start, row_count, out_channel_start) for the gwc output DMAs
GWC_BLOCKS = [(0, 16, 0), (32, 16, 16), (64, 8, 32)]
PSUM_P = 72  # psum/staging partition extent

_CACHE = {}


def _make_weights():
    """Block-diagonal mean weights, bf16: W[p, m] = 1/8 if p//8 == m."""
    ws = []
    for _, cn, _, mn in CH_TILES:
        w = np.zeros((cn, mn), dtype=np.float32)
        for p in range(cn):
            w[p, p // CPG] = 1.0 / CPG
        ws.append(w.astype(ml_dtypes.bfloat16))
    return ws


def _build_nc():
    from concourse import bacc, mybir
    import concourse.tile as tile

    f32 = mybir.dt.float32
    bf16 = mybir.dt.bfloat16

    nc = bacc.Bacc("TRN2", target_bir_lowering=False, debug=False)

    ref = nc.dram_tensor("ref_gwc", [C, HS, W], f32, kind="ExternalInput")
    tgt = nc.dram_tensor("tgt_gwc", [C, HS, W], f32, kind="ExternalInput")
    refc = nc.dram_tensor("ref_concat", [CC, HS, W], f32, kind="ExternalInput")
    tgtc = nc.dram_tensor("tgt_concat", [CC, HS, W], f32, kind="ExternalInput")
    wd = [
        nc.dram_tensor(f"w{t}", [cn, mn], bf16, kind="ExternalInput")
        for t, (_, cn, _, mn) in enumerate(CH_TILES)
    ]
    out = nc.dram_tensor("out", [COUT, D, HS, W], f32, kind="ExternalOutput")

    with tile.TileContext(nc) as tc:
        _kernel_body(nc, tc, ref, tgt, refc, tgtc, wd, out, mybir)

    nc.compile()
    return nc


def _kernel_body(nc, tc, ref, tgt, refc, tgtc, wd, out, mybir):
    f32 = mybir.dt.float32
    bf16 = mybir.dt.bfloat16
    out_ap = out.ap()

    with (
        tc.tile_pool(name="const", bufs=1) as constp,
        tc.tile_pool(name="prod", bufs=2) as prodp,
        tc.tile_pool(name="psum", bufs=2, space="PSUM") as psump,
    ):
        # --- constants / persistent buffers ---
        wt = []
        for t, (_, cn, _, mn) in enumerate(CH_TILES):
            w_t = constp.tile([cn, mn], bf16, name=f"wt{t}", tag=f"wt{t}")
            nc.sync.dma_start(w_t[:], wd[t].ap())
            wt.append(w_t)

        # concat inputs (fp32, kept in SBUF, DMA'd out per-d)
        refc_t = constp.tile([CC, HS, W], f32, name="refc_t", tag="refc_t")
        nc.sync.dma_start(refc_t[:], refc.ap())
        tgtc_t = constp.tile([CC, HS, W], f32, name="tgtc_t", tag="tgtc_t")
        nc.sync.dma_start(tgtc_t[:], tgtc.ap())

        # gwc inputs as bf16 (cast happens inside the SWDGE DMA).
        # refB holds ref shifted by one element (data at [:, :, 1:W+1],
        # row stride W+4) so odd-d views stay 4-byte aligned for DVE 2x.
        # refB is derived on-chip (ACT copy) instead of re-reading HBM.
        refA, refB, tgtT = [], [], []
        for t, (c0, cn, _, _) in enumerate(CH_TILES):
            a = constp.tile([cn, HS, W], bf16, name=f"refA{t}", tag=f"refA{t}")
            nc.gpsimd.dma_start(a[:], ref[c0:c0 + cn])
            b = constp.tile([cn, HS, W + 4], bf16, name=f"refB{t}", tag=f"refB{t}")
            nc.scalar.copy(b[:, :, 1:W + 1], a[:])
            g = constp.tile([cn, HS, W], bf16, name=f"tgtT{t}", tag=f"tgtT{t}")
            nc.gpsimd.dma_start(g[:], tgt[c0:c0 + cn])
            refA.append(a)
            refB.append(b)
            tgtT.append(g)

        # concat-channel zero strips (w < d), written as 6 rectangular
        # blocks of 8 disparities each; the per-d data DMAs re-cover
        # [d:47] afterwards (explicit dep edges enforce the order).
        zrect = constp.tile([2 * CC, 4, HS, D - 1], f32, name="zrect",
                            tag="zrect")
        nc.gpsimd.memset(zrect[:], 0.0)
        zrect_inst = {}
        for blk in range(12):
            d0 = 1 + 4 * blk
            nd = min(4, D - d0)
            inst = nc.scalar.dma_start(
                out_ap[G:COUT, d0:d0 + nd, :, 0:D - 1],
                zrect[:, 0:nd, :, :])
            for dd in range(d0, d0 + nd):
                zrect_inst[dd] = inst

        # staging buffers (manual 3-slot rotation; zeroed once, then the
        # descending-d order keeps the w<d strip zero forever)
        stg = []
        for i in range(3):
            s = constp.tile([PSUM_P, HS, W], f32, name=f"stg{i}", tag=f"stg{i}")
            nc.vector.memset(s[:], 0.0)
            stg.append(s)

        # --- main disparity loop (descending) ---
        for di, d in enumerate(reversed(range(D))):
            wv = W - d
            s = stg[di % 3]

            # products (bf16) on DVE
            prods = []
            for t, (_, cn, _, _) in enumerate(CH_TILES):
                p = prodp.tile([cn, HS, W], bf16, name=f"prod{t}_{d}",
                               tag=f"prod{t}")
                if d % 2 == 0:
                    rsrc = refA[t][0:cn, :, d:W]
                else:
                    rsrc = refB[t][0:cn, :, d + 1:W + 1]
                nc.vector.tensor_mul(p[0:cn, :, 0:wv], rsrc,
                                     tgtT[t][0:cn, :, 0:wv])
                prods.append(p)

            # group-reduce on PE, drain on ACT, one h-half at a time
            for hh in range(2):
                ps = psump.tile([PSUM_P, HS // 2, 256], f32,
                                name=f"ps_{d}_{hh}", tag="ps")
                for t, (_, cn, m0, mn) in enumerate(CH_TILES):
                    for k in range(4):
                        h0 = hh * 8 + 2 * k
                        nc.tensor.matmul(
                            ps[m0:m0 + mn, 2 * k:2 * k + 2, d:W],
                            wt[t][0:cn, 0:mn],
                            prods[t][0:cn, h0:h0 + 2, 0:wv],
                            start=True, stop=True,
                        )
                nc.scalar.copy(s[:, hh * 8:hh * 8 + 8, d:W], ps[:, :, d:W])

            # gwc channels out (full width; w<d strip is already zero)
            for p0, pn, c0 in GWC_BLOCKS:
                nc.sync.dma_start(out_ap[c0:c0 + pn, d], s[p0:p0 + pn])

            # concat channels straight from SBUF (split across the two
            # HWDGE rings; each must run after its covering zero-rect)
            i1 = nc.scalar.dma_start(out_ap[G:G + CC, d, :, d:W],
                                     refc_t[:, :, d:W])
            i2 = nc.sync.dma_start(out_ap[G + CC:COUT, d, :, d:W],
                                   tgtc_t[:, :, 0:wv])
            if d in zrect_inst:
                from concourse.bass import _add_dep_helper
                _add_dep_helper(i1.ins, zrect_inst[d].ins, sync=True,
                                reason="concat data after zero-rect")
                _add_dep_helper(i2.ins, zrect_inst[d].ins, sync=True,
                                reason="concat data after zero-rect")


def _get_nc():
    if "nc" not in _CACHE:
        _CACHE["nc"] = _build_nc()
    return _CACHE["nc"]


def kernel(ref_gwc, tgt_gwc, ref_concat, tgt_concat):
    from concourse.bass_utils import run_bass_kernel_spmd

    ref_gwc = np.asarray(ref_gwc, dtype=np.float32)
    tgt_gwc = np.asarray(tgt_gwc, dtype=np.float32)
    ref_concat = np.asarray(ref_concat, dtype=np.float32)
    tgt_concat = np.asarray(tgt_concat, dtype=np.float32)

    nc = _get_nc()
    ws = _make_weights()

    in_maps = []
    for i in range(NCORES):
        sl = slice(i * HS, (i + 1) * HS)
        m = {
            "ref_gwc": np.ascontiguousarray(ref_gwc[0, :, sl, :]),
            "tgt_gwc": np.ascontiguousarray(tgt_gwc[0, :, sl, :]),
            "ref_concat": np.ascontiguousarray(ref_concat[0, :, sl, :]),
            "tgt_concat": np.ascontiguousarray(tgt_concat[0, :, sl, :]),
        }
        for t, w in enumerate(ws):
            m[f"w{t}"] = w
        in_maps.append(m)

    res = run_bass_kernel_spmd(nc, in_maps, list(range(NCORES))).results

    full = np.empty((1, COUT, D, H, W), dtype=np.float32)
    for i in range(NCORES):
        full[0, :, :, i * HS:(i + 1) * HS, :] = res[i]["out"]
    return full



# revision 4
# speedup vs baseline: 1.6277x; 1.6277x over previous
"""GwcVolumeCostProcessor Trainium2 kernel (v2).

Builds the groupwise-correlation + concat cost volume:
  out[1, 64, 48, 128, 240] f32 from
  ref_gwc/tgt_gwc [1, 320, 128, 240] and ref_concat/tgt_concat [1, 12, 128, 240].

Sharding: H axis (128 = 8 cores x 16 rows). The disparity shift is along W
only, so each core needs just its own 16-row slice of every input.

All 64 output channels ride one pipeline. The concat channels are folded in
as pseudo-products with identity weight columns:
  - gwc groups:  prod = ref[c] * tgt[c],     weights 1/8 block-diagonal
  - ref_concat:  prod = refc[i] * ones,      weights identity (A-side slice
                 [d:W] applies the w>=d masking for free)
  - tgt_concat:  prod = ones * tgtc[i],      weights identity (S-side slice
                 [0:wv] + psum dst [d:W] applies the shift for free)

Per-core pipeline (for each disparity d, descending):
  - DVE: 3 product tiles (bf16, 2x mode)
  - PE : 3 block matmuls x 8 psum-bank chunks -> PSUM partitions 0:96
  - ACT: drains PSUM -> f32 staging (w<d strip stays zero: descending d)
  - DMA: 3 large per-d stores (16/16/32 channels x 15KB descriptors) split
         across the sync HWDGE ring, the ACT HWDGE ring, and the gpsimd
         SWDGE queue so all three DMA streams run in parallel.
"""

import numpy as np
import ml_dtypes

C = 320          # gwc channels
G = 40           # groups
CPG = 8          # channels per group
D = 48           # disparity bins
H = 128          # full height
W = 240          # width
CC = 12          # concat channels per tensor
COUT = G + 2 * CC  # 64 output channels
NCORES = 8
HS = H // NCORES  # 16 rows per core

# tiles: (gwc_ch_start, gwc_ch_count, extra_rows, psum_base, out_group_count)
# t2 carries 64 gwc channels + 12 refc rows + 12 ones rows (A side).
T0 = dict(c0=0, cn=128, rows=128, ps=0, mn=16)
T1 = dict(c0=128, cn=128, rows=128, ps=32, mn=16)
T2 = dict(c0=256, cn=64, rows=88, ps=64, mn=32)
TILES = [T0, T1, T2]
PSUM_P = 96  # psum/staging partition extent

_CACHE = {}


def _make_weights():
    """Per-tile stationary matrices, bf16."""
    w0 = np.zeros((128, 16), dtype=np.float32)
    for r in range(128):
        w0[r, r // CPG] = 1.0 / CPG
    w1 = w0.copy()
    w2 = np.zeros((88, 32), dtype=np.float32)
    for r in range(64):
        w2[r, r // CPG] = 1.0 / CPG          # gwc groups 32..39 -> cols 0..7
    for i in range(CC):
        w2[64 + i, 8 + i] = 1.0              # ref_concat -> cols 8..19
        w2[76 + i, 20 + i] = 1.0             # tgt_concat -> cols 20..31
    return [w.astype(ml_dtypes.bfloat16) for w in (w0, w1, w2)]


def _build_nc():
    from concourse import bacc, mybir
    import concourse.tile as tile

    f32 = mybir.dt.float32
    bf16 = mybir.dt.bfloat16

    nc = bacc.Bacc("TRN2", target_bir_lowering=False, debug=False)

    ref = nc.dram_tensor("ref_gwc", [C, HS, W], f32, kind="ExternalInput")
    tgt = nc.dram_tensor("tgt_gwc", [C, HS, W], f32, kind="ExternalInput")
    refc = nc.dram_tensor("ref_concat", [CC, HS, W], f32, kind="ExternalInput")
    tgtc = nc.dram_tensor("tgt_concat", [CC, HS, W], f32, kind="ExternalInput")
    wd = [
        nc.dram_tensor(f"w{t}", [ti["rows"], ti["mn"]], bf16, kind="ExternalInput")
        for t, ti in enumerate(TILES)
    ]
    out = nc.dram_tensor("out", [COUT, D, HS, W], f32, kind="ExternalOutput")

    with tile.TileContext(nc) as tc:
        _kernel_body(nc, tc, ref, tgt, refc, tgtc, wd, out, mybir)

    nc.compile()
    return nc


def _kernel_body(nc, tc, ref, tgt, refc, tgtc, wd, out, mybir):
    f32 = mybir.dt.float32
    bf16 = mybir.dt.bfloat16
    out_ap = out.ap()

    with (
        tc.tile_pool(name="const", bufs=1) as constp,
        tc.tile_pool(name="prod", bufs=2) as prodp,
        tc.tile_pool(name="psum", bufs=2, space="PSUM") as psump,
    ):
        # --- weights ---
        wt = []
        for t, ti in enumerate(TILES):
            w_t = constp.tile([ti["rows"], ti["mn"]], bf16, name=f"wt{t}",
                              tag=f"wt{t}")
            nc.sync.dma_start(w_t[:], wd[t].ap())
            wt.append(w_t)

        # --- input tiles (bf16; cast inside the SWDGE DMA) ---
        # A side (sliced [d:W] in the d-loop): ref channels (+ refc + ones)
        # B side: A shifted one element right (data at [:, :, 1:W+1]) so
        #         odd-d slices stay 4-byte aligned for DVE 2x; derived via
        #         ACT copy instead of re-reading HBM.
        # S side (sliced [0:wv]): tgt channels (+ ones + tgtc)
        refA, refB, tgtT = [], [], []
        for t, ti in enumerate(TILES):
            rows, cn, c0 = ti["rows"], ti["cn"], ti["c0"]
            a = constp.tile([rows, HS, W], bf16, name=f"refA{t}", tag=f"refA{t}")
            nc.gpsimd.dma_start(a[0:cn], ref[c0:c0 + cn])
            g = constp.tile([rows, HS, W], bf16, name=f"tgtT{t}", tag=f"tgtT{t}")
            nc.gpsimd.dma_start(g[0:cn], tgt[c0:c0 + cn])
            if rows > cn:  # t2 extras
                # memset must start 32-aligned: set ones over [64:88], then
                # the concat loads overwrite their half (WAW, program order)
                nc.vector.memset(a[64:88], 1.0)
                nc.vector.memset(g[64:88], 1.0)
                nc.gpsimd.dma_start(a[64:76], refc.ap())      # refc rows
                nc.gpsimd.dma_start(g[76:88], tgtc.ap())      # tgtc rows
            b = constp.tile([rows, HS, W + 4], bf16, name=f"refB{t}",
                            tag=f"refB{t}")
            nc.scalar.copy(b[:, :, 1:W + 1], a[:])
            refA.append(a)
            refB.append(b)
            tgtT.append(g)

        # staging buffers (3-slot rotation; zeroed once, then the
        # descending-d order keeps the w<d strip zero forever)
        stg = []
        for i in range(3):
            s = constp.tile([PSUM_P, HS, W], f32, name=f"stg{i}", tag=f"stg{i}")
            nc.vector.memset(s[:], 0.0)
            stg.append(s)

        # --- main disparity loop (descending) ---
        for di, d in enumerate(reversed(range(D))):
            wv = W - d
            s = stg[di % 3]

            # products (bf16) on DVE
            prods = []
            for t, ti in enumerate(TILES):
                rows = ti["rows"]
                p = prodp.tile([rows, HS, W], bf16, name=f"prod{t}_{d}",
                               tag=f"prod{t}")
                if d % 2 == 0:
                    rsrc = refA[t][0:rows, :, d:W]
                else:
                    rsrc = refB[t][0:rows, :, d + 1:W + 1]
                nc.vector.tensor_mul(p[0:rows, :, 0:wv], rsrc,
                                     tgtT[t][0:rows, :, 0:wv])
                prods.append(p)

            # group-reduce on PE, drain on ACT, one h-half at a time
            for hh in range(2):
                ps = psump.tile([PSUM_P, HS // 2, 256], f32,
                                name=f"ps_{d}_{hh}", tag="ps")
                for t, ti in enumerate(TILES):
                    rows, m0, mn = ti["rows"], ti["ps"], ti["mn"]
                    for k in range(4):
                        h0 = hh * 8 + 2 * k
                        nc.tensor.matmul(
                            ps[m0:m0 + mn, 2 * k:2 * k + 2, d:W],
                            wt[t][0:rows, 0:mn],
                            prods[t][0:rows, h0:h0 + 2, 0:wv],
                            start=True, stop=True,
                        )
                nc.scalar.copy(s[:, hh * 8:hh * 8 + 8, d:W], ps[:, :, d:W])

            # per-d stores: 3 large DMAs on 3 independent DMA streams
            # psum/staging partition map: 0:16 -> ch 0:16, 32:48 -> ch 16:32,
            # 64:96 -> ch 32:64 (gwc 32..39, refc, tgtc)
            nc.sync.dma_start(out_ap[0:16, d], s[0:16])
            nc.scalar.dma_start(out_ap[16:32, d], s[32:48])
            nc.gpsimd.dma_start(out_ap[32:64, d], s[64:96])


def _get_nc():
    if "nc" not in _CACHE:
        _CACHE["nc"] = _build_nc()
    return _CACHE["nc"]


def kernel(ref_gwc, tgt_gwc, ref_concat, tgt_concat):
    from concourse.bass_utils import run_bass_kernel_spmd

    ref_gwc = np.asarray(ref_gwc, dtype=np.float32)
    tgt_gwc = np.asarray(tgt_gwc, dtype=np.float32)
    ref_concat = np.asarray(ref_concat, dtype=np.float32)
    tgt_concat = np.asarray(tgt_concat, dtype=np.float32)

    nc = _get_nc()
    ws = _make_weights()

    in_maps = []
    for i in range(NCORES):
        sl = slice(i * HS, (i + 1) * HS)
        m = {
            "ref_gwc": np.ascontiguousarray(ref_gwc[0, :, sl, :]),
            "tgt_gwc": np.ascontiguousarray(tgt_gwc[0, :, sl, :]),
            "ref_concat": np.ascontiguousarray(ref_concat[0, :, sl, :]),
            "tgt_concat": np.ascontiguousarray(tgt_concat[0, :, sl, :]),
        }
        for t, w in enumerate(ws):
            m[f"w{t}"] = w
        in_maps.append(m)

    res = run_bass_kernel_spmd(nc, in_maps, list(range(NCORES))).results

    full = np.empty((1, COUT, D, H, W), dtype=np.float32)
    for i in range(NCORES):
        full[0, :, :, i * HS:(i + 1) * HS, :] = res[i]["out"]
    return full
